# revision 38
# baseline (speedup 1.0000x reference)
"""Bass/Tile kernel for nn_MCA (multi-head cross-attention), 8-core SPMD.

Sharding: batch B(4) x head-group(2) -> 8 cores. Core c handles batch
b = c//2 and heads [g*8, (g+1)*8) where g = c%2. Each core computes a
partial output (T, C) = y_g @ Wu[:, g-cols].T; host sums the two
head-group partials per batch and adds bu.

Single fused pipeline (all matmuls bf16 -> fp32 PSUM, uniform
(128,128) PE tile mode so the array never drains on a mode switch).
The exp stream on the Scalar engine is the bound (256 x [128,1024]
activations ~ 294us); every projection hides under it as paced PE
filler:

  prologue: only qT(qb0,m0) + kT(block0,m0).
  main loop over 256 groups: software-pipelined S(g+1) -> exp(g) ->
    V(g-1); filler thunks (remaining projections, prev q-block's output
    projection) drain between groups with a matmul budget.
  qb0's k-sweep is split into two half-sweeps over all head pairs
  (A: k-tiles 0-7, B: 8-15, partial o spilled to SBUF between) so the
  first sweep's new-k-tile consumption rate stays under the projection
  production rate and the exp stream never starves.
  S matmuls are zero-padded to K=128 (kTz: per-head slices, data on
  the head's partition half, zeros elsewhere).
  Rowsum via the ones-column trick (V stationary [128,65]); per head
  pair: batched DVE reciprocal, DMA hop to partition 0, gpsimd
  broadcast, DVE multiply into yT.
"""

import os
from contextlib import ExitStack

import numpy as np

_PROGRAM_CACHE = {}


def _imports():
    import concourse.bass as bass
    import concourse.tile as tile
    from concourse import bacc, mybir
    from concourse.bass_utils import run_bass_kernel_spmd

    return bass, tile, bacc, mybir, run_bass_kernel_spmd


def build_program(T=2048, C=1024, HLOC=8, n_cores=8):
    """Build + compile the per-core Tile program (SPMD; same for all cores)."""
    bass, tile, bacc, mybir, _ = _imports()
    BF16 = mybir.dt.bfloat16
    F32 = mybir.dt.float32
    AF = mybir.ActivationFunctionType
    ALU = mybir.AluOpType

    hd = 64
    DG = HLOC * hd            # head-group feature dim (512)
    P = 128
    KT = C // P               # contraction tiles for projections (8)
    MT = DG // P              # d-tiles (4)
    NBLK = 512                # t-block width for projections / q-blocks
    NB = T // NBLK            # 4
    KTT = T // P              # key tiles in attention (16)
    HP = MT                   # head pairs == d-tiles
    scale = 1.0 / np.sqrt(C)

    nc = bacc.Bacc("TRN2", target_bir_lowering=False, debug=False,
                   num_devices=n_cores)

    # host supplies pre-transposed layouts so every DMA is contiguous
    # per partition
    xqT = nc.dram_tensor("xqT", [C, T], BF16, kind="ExternalInput").ap()
    xkT = nc.dram_tensor("xkT", [C, T], BF16, kind="ExternalInput").ap()
    wqT = nc.dram_tensor("wqT", [C, DG], BF16, kind="ExternalInput").ap()
    wkT = nc.dram_tensor("wkT", [C, DG], BF16, kind="ExternalInput").ap()
    wvT = nc.dram_tensor("wvT", [C, DG], BF16, kind="ExternalInput").ap()
    wuT = nc.dram_tensor("wuT", [DG, C], BF16, kind="ExternalInput").ap()
    bq = nc.dram_tensor("bq", [P, MT], F32, kind="ExternalInput").ap()
    bk = nc.dram_tensor("bk", [P, MT], F32, kind="ExternalInput").ap()
    bv = nc.dram_tensor("bv", [1, DG], F32, kind="ExternalInput").ap()
    out = nc.dram_tensor("out", [T, C], F32, kind="ExternalOutput").ap()

    with tile.TileContext(nc) as tc, ExitStack() as ctx:
        const = ctx.enter_context(tc.tile_pool(name="const", bufs=1))
        persist = ctx.enter_context(tc.tile_pool(name="persist", bufs=1))
        xqpool = ctx.enter_context(tc.tile_pool(name="xqpool", bufs=2))
        xkpool = ctx.enter_context(tc.tile_pool(name="xkpool", bufs=3))
        # PSUM budget (8 banks): S 2x[128,2,512]=4, o 2x[65,512]=2,
        # proj 2x[128,512]=2
        ps_s = ctx.enter_context(tc.tile_pool(name="ps_s", bufs=2,
                                              space="PSUM"))
        ps_o = ctx.enter_context(tc.tile_pool(name="ps_o", bufs=2,
                                              space="PSUM"))
        ps_p = ctx.enter_context(tc.tile_pool(name="ps_p", bufs=2,
                                              space="PSUM"))
        ppool = ctx.enter_context(tc.tile_pool(name="ppool", bufs=3))
        epil = ctx.enter_context(tc.tile_pool(name="epil", bufs=4))
        outp = ctx.enter_context(tc.tile_pool(name="outp", bufs=2))

        # ---- constants / weights resident in SBUF ----
        wq_sb = persist.tile([P, KT, DG], BF16)
        wk_sb = persist.tile([P, KT, DG], BF16)
        wv_sb = persist.tile([P, KT, DG], BF16)
        wu_sb = persist.tile([P, MT, C], BF16)
        bq_sb = const.tile([P, MT], F32)
        bk_sb = const.tile([P, MT], F32)
        bv_bc = const.tile([P, DG], F32)
        # DMA in need-order: Q-projection inputs first so the PE starts
        # within a few us, output-projection weights last
        nc.sync.dma_start(out=wq_sb[:], in_=wqT.rearrange("(k p) d -> p k d", p=P))
        nc.sync.dma_start(out=bq_sb[:], in_=bq)

        # persistent activations
        qT_sb = persist.tile([P, MT, T], BF16)
        # kTz: one [128, T] slice per head, the head's 64 k-dims on its
        # partition half ((h%2)*64) and zeros on the other half, so S
        # matmuls run with K=128 (uniform tile mode, no PE drains).
        kTz_sb = persist.tile([P, HLOC, T], BF16)
        v_aug = persist.tile([P, KTT, HLOC * (hd + 1)], BF16)
        yT_sb = persist.tile([P, MT, T], BF16)

        xq_tiles = {}
        xk_tiles = {}
        proj_ps = {}

        def load_xq(nt):
            if nt not in xq_tiles:
                t = xqpool.tile([P, KT, NBLK], BF16, tag="xq", name="xq_t")
                nc.sync.dma_start(
                    out=t[:],
                    in_=xqT.rearrange("(k p) t -> p k t", p=P)[
                        :, :, nt * NBLK:(nt + 1) * NBLK])
                xq_tiles[nt] = t
            return xq_tiles[nt]

        def load_xk(nt):
            if nt not in xk_tiles:
                t = xkpool.tile([P, KT, NBLK], BF16, tag="xk", name="xk_t")
                nc.sync.dma_start(
                    out=t[:],
                    in_=xkT.rearrange("(k p) t -> p k t", p=P)[
                        :, :, nt * NBLK:(nt + 1) * NBLK])
                xk_tiles[nt] = t
            return xk_tiles[nt]

        # first x block + K-side weights next in the DMA queue
        load_xq(0)
        nc.sync.dma_start(out=wk_sb[:], in_=wkT.rearrange("(k p) d -> p k d", p=P))
        nc.sync.dma_start(out=bk_sb[:], in_=bk)
        load_xk(0)
        nc.sync.dma_start(out=wv_sb[:], in_=wvT.rearrange("(k p) d -> p k d", p=P))
        nc.sync.dma_start(out=bv_bc[:], in_=bv.partition_broadcast(P))
        nc.sync.dma_start(out=wu_sb[:], in_=wuT.rearrange("(k p) d -> p k d", p=P))

        # per-partition masks for the zero-padded kTz evictions: head 2m
        # keeps partitions 0:64, head 2m+1 keeps 64:128; the eviction
        # writes the full 128 partitions (data*mask + masked bias) so no
        # whole-tensor memset is needed (a long-running gpsimd memset
        # racing the evictions corrupts kTz).
        mask_lo = const.tile([P, 1], F32)
        mask_hi = const.tile([P, 1], F32)
        nc.vector.memset(mask_lo[0:hd, :], 1.0)
        nc.vector.memset(mask_lo[hd:P, :], 0.0)
        nc.vector.memset(mask_hi[0:hd, :], 0.0)
        nc.vector.memset(mask_hi[hd:P, :], 1.0)
        bk_lo = const.tile([P, MT], F32)
        bk_hi = const.tile([P, MT], F32)
        nc.vector.tensor_scalar(out=bk_lo[:], in0=bk_sb[:],
                                scalar1=mask_lo[:], scalar2=None,
                                op0=ALU.mult)
        nc.vector.tensor_scalar(out=bk_hi[:], in0=bk_sb[:],
                                scalar1=mask_hi[:], scalar2=None,
                                op0=ALU.mult)

        def emit_qT(nt, m, half):
            """Half of one Q-projection output tile (4 matmuls); the
            second half evicts with bias + 1/sqrt(C) scale."""
            tsl = slice(nt * NBLK, (nt + 1) * NBLK)
            xq_t = load_xq(nt)
            msl = slice(m * P, (m + 1) * P)
            if half == 0:
                proj_ps[("q", nt, m)] = ps_p.tile([P, NBLK], F32, tag="p",
                                                  name="psq")
            ps = proj_ps[("q", nt, m)]
            for k in range(4 * half, 4 * half + 4):
                nc.tensor.matmul(ps[:], wq_sb[:, k, msl], xq_t[:, k, :],
                                 start=(k == 0), stop=(k == KT - 1))
            if half == 1:
                del proj_ps[("q", nt, m)]
                nc.vector.tensor_scalar(
                    out=qT_sb[:, m, tsl], in0=ps[:],
                    scalar1=bq_sb[:, m:m + 1], scalar2=scale,
                    op0=ALU.add, op1=ALU.mult)

        def emit_kT(nt, m, half):
            """Half of one K-projection d-pair tile; the second half
            evicts into the two per-head zero-padded kTz slices."""
            tsl = slice(nt * NBLK, (nt + 1) * NBLK)
            xk_t = load_xk(nt)
            msl = slice(m * P, (m + 1) * P)
            if half == 0:
                proj_ps[("k", nt, m)] = ps_p.tile([P, NBLK], F32, tag="p",
                                                  name="psk")
            ps = proj_ps[("k", nt, m)]
            for k in range(4 * half, 4 * half + 4):
                nc.tensor.matmul(ps[:], wk_sb[:, k, msl], xk_t[:, k, :],
                                 start=(k == 0), stop=(k == KT - 1))
            if half == 1:
                del proj_ps[("k", nt, m)]
                nc.vector.tensor_scalar(
                    out=kTz_sb[:, 2 * m, tsl], in0=ps[:],
                    scalar1=mask_lo[:], scalar2=bk_lo[:, m:m + 1],
                    op0=ALU.mult, op1=ALU.add)
                nc.vector.tensor_scalar(
                    out=kTz_sb[:, 2 * m + 1, tsl], in0=ps[:],
                    scalar1=mask_hi[:], scalar2=bk_hi[:, m:m + 1],
                    op0=ALU.mult, op1=ALU.add)

        def emit_v(nt, m, half):
            """Half of one V-projection t-subtile (all DG cols)."""
            tidx = nt * (NBLK // P) + m
            msl = slice(m * P, (m + 1) * P)
            xk_t = load_xk(nt)
            if half == 0:
                proj_ps[("v", nt, m)] = ps_p.tile([P, DG], F32, tag="p",
                                                  name="psv")
            ps = proj_ps[("v", nt, m)]
            for k in range(4 * half, 4 * half + 4):
                nc.tensor.matmul(ps[:], xk_t[:, k, msl], wv_sb[:, k, :],
                                 start=(k == 0), stop=(k == KT - 1))
            if half == 1:
                del proj_ps[("v", nt, m)]
                v_row = v_aug[:, tidx]
                for h in range(HLOC):
                    nc.vector.tensor_add(
                        v_row[:, h * (hd + 1):h * (hd + 1) + hd],
                        ps[:, h * hd:(h + 1) * hd],
                        bv_bc[:, h * hd:(h + 1) * hd])
                ones_view = v_row.rearrange("p (h e) -> p h e",
                                            e=hd + 1)[:, :, hd:hd + 1]
                nc.vector.memset(ones_view, 1.0)

        def emit_outproj(qt, jt):
            """One [128 t, 512 c] tile of the output projection."""
            qsl = slice(qt * P, (qt + 1) * P)
            jsl = slice(jt * NBLK, (jt + 1) * NBLK)
            ps = ps_p.tile([P, NBLK], F32, tag="p", name="pso")
            for dt in range(MT):
                nc.tensor.matmul(ps[:], yT_sb[:, dt, qsl],
                                 wu_sb[:, dt, jsl],
                                 start=(dt == 0), stop=(dt == MT - 1))
            o_sb = outp.tile([P, NBLK], F32, tag="osb", name="osb")
            nc.vector.tensor_copy(o_sb[:], ps[:])
            nc.sync.dma_start(out=out[qsl, jsl], in_=o_sb[:])

        # ---- filler stream: (n_matmuls, thunk) pairs, paced by budget ----
        filler = []

        def drain_filler(mm_budget):
            while mm_budget > 0 and filler:
                n_mm, fn = filler.pop(0)
                fn()
                mm_budget -= n_mm

        def q_thunk(fn, *args):
            filler.append((4, lambda a=args: fn(*a)))

        def queue_kT(nt, m):
            for half in range(2):
                q_thunk(emit_kT, nt, m, half)

        def queue_qT_m(nt, m):
            for half in range(2):
                q_thunk(emit_qT, nt, m, half)

        def queue_v_block(nt):
            for m in range(NBLK // P):
                for half in range(2):
                    q_thunk(emit_v, nt, m, half)

        def queue_outproj(qb):
            for qt in range(qb * NBLK // P, (qb + 1) * NBLK // P):
                for jt in range(C // NBLK):
                    filler.append(
                        (4, lambda qt=qt, jt=jt: emit_outproj(qt, jt)))

        # ---- prologue ----
        # The PE pulls LDWEIGHTS up to ~64 instructions ahead of in-flight
        # matmuls, so stationary operands (kTz, v_aug) must be WRITTEN with
        # that much emission distance before their first consuming matmul.
        # Prologue covers the first half-sweep's k-tiles generously.
        for half in range(2):
            emit_qT(0, 0, half)
        for nt in (0, 1):
            for half in range(2):
                emit_kT(nt, 0, half)
        for nt in (0, 1):
            for m in range(NBLK // P):
                for half in range(2):
                    emit_v(nt, m, half)

        # deadline-ordered filler for the A/B q-block-0 schedule: the A
        # half-sweep (k-tiles 0-7, blocks 0-1 in prologue) needs m1-3 +
        # qT0 m1-3 before hp1/2/3 start (g=8/16/24); the B half-sweep
        # (g>=32) consumes blocks 2-3.
        for m in range(1, MT):
            queue_kT(0, m)
            queue_kT(1, m)
            queue_qT_m(0, m)
        queue_kT(2, 0)
        queue_kT(3, 0)
        queue_v_block(2)
        queue_v_block(3)
        for m in range(1, MT):
            queue_kT(2, m)
            queue_kT(3, m)


        # ---- group list ----
        # entry: (qb, hp, h2, ktp, lo, hi, final)
        groups = []
        qb_first_group = set()
        for qb in range(NB):
            qb_first_group.add(len(groups))
            phases = [(0, 4, False), (4, 8, True)] if qb == 0 else [(0, 8, True)]
            for lo, hi, final in phases:
                for hp in range(HP):
                    for ktp in range(lo, hi):
                        for h2 in range(2):
                            groups.append((qb, hp, h2, ktp, lo, hi, final))
        NG = len(groups)

        def emit_S(g):
            qb, hp, h2, ktp, lo, hi, final = groups[g]
            h = 2 * hp + h2
            qsl = slice(qb * NBLK, (qb + 1) * NBLK)
            s = ps_s.tile([P, 2, NBLK], F32, tag="s", name="s_ps")
            for j in range(2):
                kt = 2 * ktp + j
                ksl = slice(kt * P, (kt + 1) * P)
                nc.tensor.matmul(s[:, j, :], kTz_sb[:, h, ksl],
                                 qT_sb[:, hp, qsl], start=True, stop=True)
            return s

        def emit_exp(g, s):
            p = ppool.tile([P, 2, NBLK], BF16, tag="p", name="p_sb")
            nc.scalar.activation(p[:], s[:], AF.Exp)
            return p

        o_tiles = {}    # (qb, h) -> psum tile accumulating [65, NBLK]
        acc_tiles = {}  # (qb, h) -> SBUF partial o from the A half-sweep
        norm_sbs = {}   # (qb, h) -> o_sb copy awaiting the pair recip
        rs_tiles = {}   # (qb, hp) -> [2, NBLK] gathered rowsums

        def emit_norm_copy(qb, h, o_t):
            """Evict o psum (adding the A-phase partial if any); gather the
            rowsum row into the per-pair batch via DMA."""
            hp = h // 2
            if (qb, hp) not in rs_tiles:
                rs_tiles[(qb, hp)] = epil.tile([2, NBLK], F32, tag="rs",
                                               bufs=2, name="rs_all")
            o_sb = epil.tile([hd + 1, NBLK], F32, tag="o_sb", bufs=3,
                             name="o_sb")
            if (qb, h) in acc_tiles:
                nc.vector.tensor_add(o_sb[:], o_t[:],
                                     acc_tiles.pop((qb, h))[:])
            else:
                nc.vector.tensor_copy(o_sb[:], o_t[:])
            nc.sync.dma_start(out=rs_tiles[(qb, hp)][h % 2:h % 2 + 1, :],
                              in_=o_sb[hd:hd + 1, :])
            norm_sbs[(qb, h)] = o_sb

        def emit_norm_finish(qb, hp):
            """Pair reciprocal, then per-head broadcast + multiply into
            yT."""
            qsl = slice(qb * NBLK, (qb + 1) * NBLK)
            recip_t = epil.tile([2, NBLK], F32, tag="recip", bufs=2,
                                name="recip_t")
            nc.vector.reciprocal(recip_t[:], rs_tiles.pop((qb, hp))[:])
            for h2 in range(2):
                h = 2 * hp + h2
                hrow = slice(0, hd) if h2 == 0 else slice(hd, P)
                rtmp = epil.tile([1, NBLK], F32, tag="rtmp", bufs=2,
                                 name="rtmp")
                nc.sync.dma_start(out=rtmp[:], in_=recip_t[h2:h2 + 1, :])
                bcast = epil.tile([hd, NBLK], F32, tag="bcast", bufs=2,
                                  name="bcast")
                nc.gpsimd.partition_broadcast(bcast[:], rtmp[:])
                nc.vector.tensor_mul(yT_sb[hrow, hp, qsl],
                                     norm_sbs.pop((qb, h))[0:hd, :],
                                     bcast[:])

        def emit_V(g, p):
            qb, hp, h2, ktp, lo, hi, final = groups[g]
            h = 2 * hp + h2
            key = (qb, h)
            if ktp == lo:
                o_tiles[key] = ps_o.tile([hd + 1, NBLK], F32, tag="o",
                                         name="o_ps")
            o_t = o_tiles[key]
            for j in range(2):
                kt = 2 * ktp + j
                nc.tensor.matmul(
                    o_t[:], v_aug[:, kt, h * (hd + 1):(h + 1) * (hd + 1)],
                    p[:, j, :], start=(ktp == lo and j == 0),
                    stop=(ktp == hi - 1 and j == 1))
            if ktp == hi - 1:
                o_t = o_tiles.pop(key)
                if not final:
                    acc = epil.tile([hd + 1, NBLK], F32, tag="oacc",
                                    bufs=8, name="oacc")
                    nc.vector.tensor_copy(acc[:], o_t[:])
                    acc_tiles[key] = acc
                else:
                    emit_norm_copy(qb, h, o_t)
                    if h2 == 1:
                        emit_norm_finish(qb, hp)

        # software pipeline: S(g+1) ahead of exp(g) ahead of V(g-1)
        s_tiles = {0: emit_S(0)}
        p_tiles = {}
        for g in range(NG):
            qb, hp, h2, ktp, lo, hi, final = groups[g]
            if g in qb_first_group:
                if qb + 1 < NB:
                    for m in range(MT):
                        queue_qT_m(qb + 1, m)
                if qb >= 1:
                    queue_outproj(qb - 1)
            if g + 1 < NG:
                s_tiles[g + 1] = emit_S(g + 1)
            p_tiles[g] = emit_exp(g, s_tiles.pop(g))
            if g - 1 >= 0:
                emit_V(g - 1, p_tiles.pop(g - 1))
            # pace the filler: the first half-sweep is production-bound on
            # the kT/v projections, so drain aggressively there; then keep
            # ~2 matmuls per group so the exp stream is never starved
            drain_filler(16 if g < 20 else (4 if g < 64 else 2))
        emit_V(NG - 1, p_tiles.pop(NG - 1))

        # ---- epilogue: last q-block's output projection + leftovers ----
        queue_outproj(NB - 1)
        drain_filler(10 ** 9)

    nc.compile()
    return nc


def _get_program():
    key = "main"
    if key not in _PROGRAM_CACHE:
        _PROGRAM_CACHE[key] = build_program()
    return _PROGRAM_CACHE[key]


def make_in_maps(x1, x2, Wq, bq, Wk, bk, Wv, bv, Wu, bu, n_cores=8):
    import ml_dtypes
    bf16 = ml_dtypes.bfloat16
    T, B, C = x1.shape
    DG = C // 2  # head-group feature dim (8 heads x 64)
    P, KT, NBLK, NB = 128, C // 128, 512, T // 512
    x1 = np.asarray(x1, np.float32)
    x2 = np.asarray(x2, np.float32)

    def prep_w(w):  # [C, M] -> [128, KT, M]: partition p <- row k*128+p
        return np.ascontiguousarray(
            w.reshape(KT, P, -1).transpose(1, 0, 2)).astype(bf16)

    def prep_x(xT):  # [C, T] -> [NB, 128, KT, NBLK]
        return np.ascontiguousarray(
            xT.reshape(KT, P, NB, NBLK).transpose(2, 1, 0, 3)).astype(bf16)

    in_maps = []
    for core in range(n_cores):
        b, g = core // 2, core % 2
        gs = slice(g * DG, (g + 1) * DG)
        in_maps.append({
            "xqT": np.ascontiguousarray(x1[:, b, :].T).astype(bf16),
            "xkT": np.ascontiguousarray(x2[:, b, :].T).astype(bf16),
            "wqT": np.ascontiguousarray(np.asarray(Wq, np.float32)[gs, :].T).astype(bf16),
            "wkT": np.ascontiguousarray(np.asarray(Wk, np.float32)[gs, :].T).astype(bf16),
            "wvT": np.ascontiguousarray(np.asarray(Wv, np.float32)[gs, :].T).astype(bf16),
            "wuT": np.ascontiguousarray(np.asarray(Wu, np.float32)[:, gs].T).astype(bf16),
            "bq": np.ascontiguousarray(
                np.asarray(bq, np.float32)[gs].reshape(-1, P).T),
            "bk": np.ascontiguousarray(
                np.asarray(bk, np.float32)[gs].reshape(-1, P).T),
            "bv": np.asarray(bv, np.float32)[gs].reshape(1, DG),
        })
    return in_maps


def kernel(x1, x2, Wq, bq, Wk, bk, Wv, bv, Wu, bu, _results_hook=None):
    _, _, _, _, run_bass_kernel_spmd = _imports()
    T, B, C = x1.shape
    nc = _get_program()
    in_maps = make_in_maps(x1, x2, Wq, bq, Wk, bk, Wv, bv, Wu, bu)
    br = run_bass_kernel_spmd(nc, in_maps, list(range(8)))
    if _results_hook is not None:
        _results_hook(br)
    outs = [np.asarray(r["out"], np.float32) for r in br.results]
    bu = np.asarray(bu, np.float32)
    full = np.stack([outs[2 * b] + outs[2 * b + 1] for b in range(B)], axis=0)
    full += bu.reshape(1, 1, -1)
    return full.astype(np.float32)


# revision 39
# speedup vs baseline: 1.0033x; 1.0033x over previous
"""Bass/Tile kernel for nn_MCA (multi-head cross-attention), 8-core SPMD.

Sharding: batch B(4) x head-group(2) -> 8 cores. Core c handles batch
b = c//2 and heads [g*8, (g+1)*8) where g = c%2. Each core computes a
partial output (T, C) = y_g @ Wu[:, g-cols].T; host sums the two
head-group partials per batch and adds bu.

Single fused pipeline (all matmuls bf16 -> fp32 PSUM, uniform
(128,128) PE tile mode so the array never drains on a mode switch).
The exp stream on the Scalar engine is the bound (256 x [128,1024]
activations ~ 294us); every projection hides under it as paced PE
filler:

  prologue: only qT(qb0,m0) + kT(block0,m0).
  main loop over 256 groups: software-pipelined S(g+1) -> exp(g) ->
    V(g-1); filler thunks (remaining projections, prev q-block's output
    projection) drain between groups with a matmul budget.
  qb0's k-sweep is split into two half-sweeps over all head pairs
  (A: k-tiles 0-7, B: 8-15, partial o spilled to SBUF between) so the
  first sweep's new-k-tile consumption rate stays under the projection
  production rate and the exp stream never starves.
  S matmuls are zero-padded to K=128 (kTz: per-head slices, data on
  the head's partition half, zeros elsewhere).
  Rowsum via the ones-column trick (V stationary [128,65]); per head
  pair: batched DVE reciprocal, DMA hop to partition 0, gpsimd
  broadcast, DVE multiply into yT.
"""

import os
from contextlib import ExitStack

import numpy as np

_PROGRAM_CACHE = {}


def _imports():
    import concourse.bass as bass
    import concourse.tile as tile
    from concourse import bacc, mybir
    from concourse.bass_utils import run_bass_kernel_spmd

    return bass, tile, bacc, mybir, run_bass_kernel_spmd


def build_program(T=2048, C=1024, HLOC=8, n_cores=8):
    """Build + compile the per-core Tile program (SPMD; same for all cores)."""
    bass, tile, bacc, mybir, _ = _imports()
    BF16 = mybir.dt.bfloat16
    F32 = mybir.dt.float32
    AF = mybir.ActivationFunctionType
    ALU = mybir.AluOpType

    hd = 64
    DG = HLOC * hd            # head-group feature dim (512)
    P = 128
    KT = C // P               # contraction tiles for projections (8)
    MT = DG // P              # d-tiles (4)
    NBLK = 512                # t-block width for projections / q-blocks
    NB = T // NBLK            # 4
    KTT = T // P              # key tiles in attention (16)
    HP = MT                   # head pairs == d-tiles
    scale = 1.0 / np.sqrt(C)

    nc = bacc.Bacc("TRN2", target_bir_lowering=False, debug=False,
                   num_devices=n_cores)

    # host supplies pre-transposed layouts so every DMA is contiguous
    # per partition
    xqT = nc.dram_tensor("xqT", [C, T], BF16, kind="ExternalInput").ap()
    xkT = nc.dram_tensor("xkT", [C, T], BF16, kind="ExternalInput").ap()
    wqT = nc.dram_tensor("wqT", [C, DG], BF16, kind="ExternalInput").ap()
    wkT = nc.dram_tensor("wkT", [C, DG], BF16, kind="ExternalInput").ap()
    wvT = nc.dram_tensor("wvT", [C, DG], BF16, kind="ExternalInput").ap()
    wuT = nc.dram_tensor("wuT", [DG, C], BF16, kind="ExternalInput").ap()
    bq = nc.dram_tensor("bq", [P, MT], F32, kind="ExternalInput").ap()
    bk = nc.dram_tensor("bk", [P, MT], F32, kind="ExternalInput").ap()
    bv = nc.dram_tensor("bv", [1, DG], F32, kind="ExternalInput").ap()
    out = nc.dram_tensor("out", [T, C], F32, kind="ExternalOutput").ap()

    with tile.TileContext(nc) as tc, ExitStack() as ctx:
        const = ctx.enter_context(tc.tile_pool(name="const", bufs=1))
        persist = ctx.enter_context(tc.tile_pool(name="persist", bufs=1))
        xqpool = ctx.enter_context(tc.tile_pool(name="xqpool", bufs=2))
        xkpool = ctx.enter_context(tc.tile_pool(name="xkpool", bufs=3))
        # PSUM budget (8 banks): S 2x[128,2,512]=4, o 2x[65,512]=2,
        # proj 2x[128,512]=2
        ps_s = ctx.enter_context(tc.tile_pool(name="ps_s", bufs=2,
                                              space="PSUM"))
        ps_o = ctx.enter_context(tc.tile_pool(name="ps_o", bufs=2,
                                              space="PSUM"))
        ps_p = ctx.enter_context(tc.tile_pool(name="ps_p", bufs=2,
                                              space="PSUM"))
        ppool = ctx.enter_context(tc.tile_pool(name="ppool", bufs=3))
        epil = ctx.enter_context(tc.tile_pool(name="epil", bufs=4))
        outp = ctx.enter_context(tc.tile_pool(name="outp", bufs=2))

        # ---- constants / weights resident in SBUF ----
        wq_sb = persist.tile([P, KT, DG], BF16)
        wk_sb = persist.tile([P, KT, DG], BF16)
        wv_sb = persist.tile([P, KT, DG], BF16)
        wu_sb = persist.tile([P, MT, C], BF16)
        bq_sb = const.tile([P, MT], F32)
        bk_sb = const.tile([P, MT], F32)
        bv_bc = const.tile([P, DG], F32)
        # DMA in need-order: Q-projection inputs first so the PE starts
        # within a few us, output-projection weights last
        nc.sync.dma_start(out=wq_sb[:], in_=wqT.rearrange("(k p) d -> p k d", p=P))
        nc.sync.dma_start(out=bq_sb[:], in_=bq)

        # persistent activations
        qT_sb = persist.tile([P, MT, T], BF16)
        # kTz: one [128, T] slice per head, the head's 64 k-dims on its
        # partition half ((h%2)*64) and zeros on the other half, so S
        # matmuls run with K=128 (uniform tile mode, no PE drains).
        kTz_sb = persist.tile([P, HLOC, T], BF16)
        v_aug = persist.tile([P, KTT, HLOC * (hd + 1)], BF16)
        yT_sb = persist.tile([P, MT, T], BF16)

        xq_tiles = {}
        xk_tiles = {}
        proj_ps = {}

        def load_xq(nt):
            if nt not in xq_tiles:
                t = xqpool.tile([P, KT, NBLK], BF16, tag="xq", name="xq_t")
                nc.sync.dma_start(
                    out=t[:],
                    in_=xqT.rearrange("(k p) t -> p k t", p=P)[
                        :, :, nt * NBLK:(nt + 1) * NBLK])
                xq_tiles[nt] = t
            return xq_tiles[nt]

        def load_xk(nt):
            if nt not in xk_tiles:
                t = xkpool.tile([P, KT, NBLK], BF16, tag="xk", name="xk_t")
                nc.sync.dma_start(
                    out=t[:],
                    in_=xkT.rearrange("(k p) t -> p k t", p=P)[
                        :, :, nt * NBLK:(nt + 1) * NBLK])
                xk_tiles[nt] = t
            return xk_tiles[nt]

        # first x block + K-side weights next in the DMA queue
        load_xq(0)
        nc.sync.dma_start(out=wk_sb[:], in_=wkT.rearrange("(k p) d -> p k d", p=P))
        nc.sync.dma_start(out=bk_sb[:], in_=bk)
        load_xk(0)
        nc.sync.dma_start(out=wv_sb[:], in_=wvT.rearrange("(k p) d -> p k d", p=P))
        nc.sync.dma_start(out=bv_bc[:], in_=bv.partition_broadcast(P))
        nc.sync.dma_start(out=wu_sb[:], in_=wuT.rearrange("(k p) d -> p k d", p=P))

        # per-partition masks for the zero-padded kTz evictions: head 2m
        # keeps partitions 0:64, head 2m+1 keeps 64:128; the eviction
        # writes the full 128 partitions (data*mask + masked bias) so no
        # whole-tensor memset is needed (a long-running gpsimd memset
        # racing the evictions corrupts kTz).
        mask_lo = const.tile([P, 1], F32)
        mask_hi = const.tile([P, 1], F32)
        nc.vector.memset(mask_lo[0:hd, :], 1.0)
        nc.vector.memset(mask_lo[hd:P, :], 0.0)
        nc.vector.memset(mask_hi[0:hd, :], 0.0)
        nc.vector.memset(mask_hi[hd:P, :], 1.0)
        bk_lo = const.tile([P, MT], F32)
        bk_hi = const.tile([P, MT], F32)
        nc.vector.tensor_scalar(out=bk_lo[:], in0=bk_sb[:],
                                scalar1=mask_lo[:], scalar2=None,
                                op0=ALU.mult)
        nc.vector.tensor_scalar(out=bk_hi[:], in0=bk_sb[:],
                                scalar1=mask_hi[:], scalar2=None,
                                op0=ALU.mult)

        def emit_qT(nt, m, half):
            """Half of one Q-projection output tile (4 matmuls); the
            second half evicts with bias + 1/sqrt(C) scale."""
            tsl = slice(nt * NBLK, (nt + 1) * NBLK)
            xq_t = load_xq(nt)
            msl = slice(m * P, (m + 1) * P)
            if half == 0:
                proj_ps[("q", nt, m)] = ps_p.tile([P, NBLK], F32, tag="p",
                                                  name="psq")
            ps = proj_ps[("q", nt, m)]
            for k in range(4 * half, 4 * half + 4):
                nc.tensor.matmul(ps[:], wq_sb[:, k, msl], xq_t[:, k, :],
                                 start=(k == 0), stop=(k == KT - 1))
            if half == 1:
                del proj_ps[("q", nt, m)]
                nc.vector.tensor_scalar(
                    out=qT_sb[:, m, tsl], in0=ps[:],
                    scalar1=bq_sb[:, m:m + 1], scalar2=scale,
                    op0=ALU.add, op1=ALU.mult)

        def emit_kT(nt, m, half):
            """Half of one K-projection d-pair tile; the second half
            evicts into the two per-head zero-padded kTz slices."""
            tsl = slice(nt * NBLK, (nt + 1) * NBLK)
            xk_t = load_xk(nt)
            msl = slice(m * P, (m + 1) * P)
            if half == 0:
                proj_ps[("k", nt, m)] = ps_p.tile([P, NBLK], F32, tag="p",
                                                  name="psk")
            ps = proj_ps[("k", nt, m)]
            for k in range(4 * half, 4 * half + 4):
                nc.tensor.matmul(ps[:], wk_sb[:, k, msl], xk_t[:, k, :],
                                 start=(k == 0), stop=(k == KT - 1))
            if half == 1:
                del proj_ps[("k", nt, m)]
                nc.vector.tensor_scalar(
                    out=kTz_sb[:, 2 * m, tsl], in0=ps[:],
                    scalar1=mask_lo[:], scalar2=bk_lo[:, m:m + 1],
                    op0=ALU.mult, op1=ALU.add)
                nc.vector.tensor_scalar(
                    out=kTz_sb[:, 2 * m + 1, tsl], in0=ps[:],
                    scalar1=mask_hi[:], scalar2=bk_hi[:, m:m + 1],
                    op0=ALU.mult, op1=ALU.add)

        def emit_v(nt, m, half):
            """Half of one V-projection t-subtile (all DG cols)."""
            tidx = nt * (NBLK // P) + m
            msl = slice(m * P, (m + 1) * P)
            xk_t = load_xk(nt)
            if half == 0:
                proj_ps[("v", nt, m)] = ps_p.tile([P, DG], F32, tag="p",
                                                  name="psv")
            ps = proj_ps[("v", nt, m)]
            for k in range(4 * half, 4 * half + 4):
                nc.tensor.matmul(ps[:], xk_t[:, k, msl], wv_sb[:, k, :],
                                 start=(k == 0), stop=(k == KT - 1))
            if half == 1:
                del proj_ps[("v", nt, m)]
                v_row = v_aug[:, tidx]
                for h in range(HLOC):
                    nc.vector.tensor_add(
                        v_row[:, h * (hd + 1):h * (hd + 1) + hd],
                        ps[:, h * hd:(h + 1) * hd],
                        bv_bc[:, h * hd:(h + 1) * hd])
                ones_view = v_row.rearrange("p (h e) -> p h e",
                                            e=hd + 1)[:, :, hd:hd + 1]
                nc.vector.memset(ones_view, 1.0)

        def emit_outproj(qt, jt):
            """One [128 t, 512 c] tile of the output projection."""
            qsl = slice(qt * P, (qt + 1) * P)
            jsl = slice(jt * NBLK, (jt + 1) * NBLK)
            ps = ps_p.tile([P, NBLK], F32, tag="p", name="pso")
            for dt in range(MT):
                nc.tensor.matmul(ps[:], yT_sb[:, dt, qsl],
                                 wu_sb[:, dt, jsl],
                                 start=(dt == 0), stop=(dt == MT - 1))
            o_sb = outp.tile([P, NBLK], F32, tag="osb", name="osb")
            nc.vector.tensor_copy(o_sb[:], ps[:])
            nc.sync.dma_start(out=out[qsl, jsl], in_=o_sb[:])

        # ---- filler stream: (n_matmuls, thunk) pairs, paced by budget ----
        filler = []

        def drain_filler(mm_budget):
            while mm_budget > 0 and filler:
                n_mm, fn = filler.pop(0)
                fn()
                mm_budget -= n_mm

        def q_thunk(fn, *args):
            filler.append((4, lambda a=args: fn(*a)))

        def queue_kT(nt, m):
            for half in range(2):
                q_thunk(emit_kT, nt, m, half)

        def queue_qT_m(nt, m):
            for half in range(2):
                q_thunk(emit_qT, nt, m, half)

        def queue_v_block(nt):
            for m in range(NBLK // P):
                for half in range(2):
                    q_thunk(emit_v, nt, m, half)

        def queue_outproj(qb):
            for qt in range(qb * NBLK // P, (qb + 1) * NBLK // P):
                for jt in range(C // NBLK):
                    filler.append(
                        (4, lambda qt=qt, jt=jt: emit_outproj(qt, jt)))

        # ---- prologue ----
        # The PE pulls LDWEIGHTS up to ~64 instructions ahead of in-flight
        # matmuls, so stationary operands (kTz, v_aug) must be WRITTEN with
        # that much emission distance before their first consuming matmul.
        # Prologue covers the first half-sweep's k-tiles generously.
        for half in range(2):
            emit_qT(0, 0, half)
        for nt in (0, 1):
            for half in range(2):
                emit_kT(nt, 0, half)
        for nt in (0, 1):
            for m in range(NBLK // P):
                for half in range(2):
                    emit_v(nt, m, half)

        # deadline-ordered filler for the A/B q-block-0 schedule: m-sets
        # for blocks 0-1 + qT0 lead hp1/2/3 of the A half-sweep (g=8/16/
        # 24); blocks 2-3 m0 + v2/v3 lead the B half-sweep (g>=32); late
        # m-slices of blocks 2-3 lead their B head pairs (g=40/48/56);
        # qT(qb1) by g=63. The budget paces these so the exp stream is fed
        # while the surplus spills into qb1-3 where the PE has spare time.
        for m in range(1, MT):
            queue_kT(0, m)
            queue_kT(1, m)
            queue_qT_m(0, m)
        queue_kT(2, 0)
        queue_kT(3, 0)
        queue_v_block(2)
        queue_v_block(3)
        queue_kT(2, 1)
        queue_kT(3, 1)
        for m in range(MT):
            queue_qT_m(1, m)
        for m in range(2, MT):
            queue_kT(2, m)
            queue_kT(3, m)


        # ---- group list ----
        # entry: (qb, hp, h2, ktp, lo, hi, final)
        groups = []
        qb_first_group = set()
        for qb in range(NB):
            qb_first_group.add(len(groups))
            phases = [(0, 4, False), (4, 8, True)] if qb == 0 else [(0, 8, True)]
            for lo, hi, final in phases:
                for hp in range(HP):
                    for ktp in range(lo, hi):
                        for h2 in range(2):
                            groups.append((qb, hp, h2, ktp, lo, hi, final))
        NG = len(groups)

        def emit_S(g):
            qb, hp, h2, ktp, lo, hi, final = groups[g]
            h = 2 * hp + h2
            qsl = slice(qb * NBLK, (qb + 1) * NBLK)
            s = ps_s.tile([P, 2, NBLK], F32, tag="s", name="s_ps")
            for j in range(2):
                kt = 2 * ktp + j
                ksl = slice(kt * P, (kt + 1) * P)
                nc.tensor.matmul(s[:, j, :], kTz_sb[:, h, ksl],
                                 qT_sb[:, hp, qsl], start=True, stop=True)
            return s

        def emit_exp(g, s):
            p = ppool.tile([P, 2, NBLK], BF16, tag="p", name="p_sb")
            nc.scalar.activation(p[:], s[:], AF.Exp)
            return p

        o_tiles = {}    # (qb, h) -> psum tile accumulating [65, NBLK]
        acc_tiles = {}  # (qb, h) -> SBUF partial o from the A half-sweep
        norm_sbs = {}   # (qb, h) -> o_sb copy awaiting the pair recip
        rs_tiles = {}   # (qb, hp) -> [2, NBLK] gathered rowsums

        def emit_norm_copy(qb, h, o_t):
            """Evict o psum (adding the A-phase partial if any); gather the
            rowsum row into the per-pair batch via DMA."""
            hp = h // 2
            if (qb, hp) not in rs_tiles:
                rs_tiles[(qb, hp)] = epil.tile([2, NBLK], F32, tag="rs",
                                               bufs=2, name="rs_all")
            o_sb = epil.tile([hd + 1, NBLK], F32, tag="o_sb", bufs=3,
                             name="o_sb")
            if (qb, h) in acc_tiles:
                nc.vector.tensor_add(o_sb[:], o_t[:],
                                     acc_tiles.pop((qb, h))[:])
            else:
                nc.vector.tensor_copy(o_sb[:], o_t[:])
            nc.sync.dma_start(out=rs_tiles[(qb, hp)][h % 2:h % 2 + 1, :],
                              in_=o_sb[hd:hd + 1, :])
            norm_sbs[(qb, h)] = o_sb

        def emit_norm_finish(qb, hp):
            """Pair reciprocal, then per-head broadcast + multiply into
            yT."""
            qsl = slice(qb * NBLK, (qb + 1) * NBLK)
            recip_t = epil.tile([2, NBLK], F32, tag="recip", bufs=2,
                                name="recip_t")
            nc.vector.reciprocal(recip_t[:], rs_tiles.pop((qb, hp))[:])
            for h2 in range(2):
                h = 2 * hp + h2
                hrow = slice(0, hd) if h2 == 0 else slice(hd, P)
                rtmp = epil.tile([1, NBLK], F32, tag="rtmp", bufs=2,
                                 name="rtmp")
                nc.sync.dma_start(out=rtmp[:], in_=recip_t[h2:h2 + 1, :])
                bcast = epil.tile([hd, NBLK], F32, tag="bcast", bufs=2,
                                  name="bcast")
                nc.gpsimd.partition_broadcast(bcast[:], rtmp[:])
                nc.vector.tensor_mul(yT_sb[hrow, hp, qsl],
                                     norm_sbs.pop((qb, h))[0:hd, :],
                                     bcast[:])

        def emit_V(g, p):
            qb, hp, h2, ktp, lo, hi, final = groups[g]
            h = 2 * hp + h2
            key = (qb, h)
            if ktp == lo:
                o_tiles[key] = ps_o.tile([hd + 1, NBLK], F32, tag="o",
                                         name="o_ps")
            o_t = o_tiles[key]
            for j in range(2):
                kt = 2 * ktp + j
                nc.tensor.matmul(
                    o_t[:], v_aug[:, kt, h * (hd + 1):(h + 1) * (hd + 1)],
                    p[:, j, :], start=(ktp == lo and j == 0),
                    stop=(ktp == hi - 1 and j == 1))
            if ktp == hi - 1:
                o_t = o_tiles.pop(key)
                if not final:
                    acc = epil.tile([hd + 1, NBLK], F32, tag="oacc",
                                    bufs=8, name="oacc")
                    nc.vector.tensor_copy(acc[:], o_t[:])
                    acc_tiles[key] = acc
                else:
                    emit_norm_copy(qb, h, o_t)
                    if h2 == 1:
                        emit_norm_finish(qb, hp)

        # software pipeline: S(g+1) ahead of exp(g) ahead of V(g-1)
        s_tiles = {0: emit_S(0)}
        p_tiles = {}
        for g in range(NG):
            qb, hp, h2, ktp, lo, hi, final = groups[g]
            if g in qb_first_group:
                if 1 <= qb + 1 < NB and qb >= 1:
                    for m in range(MT):
                        queue_qT_m(qb + 1, m)
                if qb >= 1:
                    queue_outproj(qb - 1)
            if g + 1 < NG:
                s_tiles[g + 1] = emit_S(g + 1)
            p_tiles[g] = emit_exp(g, s_tiles.pop(g))
            if g - 1 >= 0:
                emit_V(g - 1, p_tiles.pop(g - 1))
            # deadline-paced filler drain
            if g < 8:
                b = 4
            elif g < 24:
                b = 7
            elif g < 64:
                b = 4
            else:
                b = 3
            drain_filler(b)
        emit_V(NG - 1, p_tiles.pop(NG - 1))

        # ---- epilogue: last q-block's output projection + leftovers ----
        queue_outproj(NB - 1)
        drain_filler(10 ** 9)

    nc.compile()
    return nc


def _get_program():
    key = "main"
    if key not in _PROGRAM_CACHE:
        _PROGRAM_CACHE[key] = build_program()
    return _PROGRAM_CACHE[key]


def make_in_maps(x1, x2, Wq, bq, Wk, bk, Wv, bv, Wu, bu, n_cores=8):
    import ml_dtypes
    bf16 = ml_dtypes.bfloat16
    T, B, C = x1.shape
    DG = C // 2  # head-group feature dim (8 heads x 64)
    P, KT, NBLK, NB = 128, C // 128, 512, T // 512
    x1 = np.asarray(x1, np.float32)
    x2 = np.asarray(x2, np.float32)

    def prep_w(w):  # [C, M] -> [128, KT, M]: partition p <- row k*128+p
        return np.ascontiguousarray(
            w.reshape(KT, P, -1).transpose(1, 0, 2)).astype(bf16)

    def prep_x(xT):  # [C, T] -> [NB, 128, KT, NBLK]
        return np.ascontiguousarray(
            xT.reshape(KT, P, NB, NBLK).transpose(2, 1, 0, 3)).astype(bf16)

    in_maps = []
    for core in range(n_cores):
        b, g = core // 2, core % 2
        gs = slice(g * DG, (g + 1) * DG)
        in_maps.append({
            "xqT": np.ascontiguousarray(x1[:, b, :].T).astype(bf16),
            "xkT": np.ascontiguousarray(x2[:, b, :].T).astype(bf16),
            "wqT": np.ascontiguousarray(np.asarray(Wq, np.float32)[gs, :].T).astype(bf16),
            "wkT": np.ascontiguousarray(np.asarray(Wk, np.float32)[gs, :].T).astype(bf16),
            "wvT": np.ascontiguousarray(np.asarray(Wv, np.float32)[gs, :].T).astype(bf16),
            "wuT": np.ascontiguousarray(np.asarray(Wu, np.float32)[:, gs].T).astype(bf16),
            "bq": np.ascontiguousarray(
                np.asarray(bq, np.float32)[gs].reshape(-1, P).T),
            "bk": np.ascontiguousarray(
                np.asarray(bk, np.float32)[gs].reshape(-1, P).T),
            "bv": np.asarray(bv, np.float32)[gs].reshape(1, DG),
        })
    return in_maps


def kernel(x1, x2, Wq, bq, Wk, bk, Wv, bv, Wu, bu, _results_hook=None):
    _, _, _, _, run_bass_kernel_spmd = _imports()
    T, B, C = x1.shape
    nc = _get_program()
    in_maps = make_in_maps(x1, x2, Wq, bq, Wk, bk, Wv, bv, Wu, bu)
    br = run_bass_kernel_spmd(nc, in_maps, list(range(8)))
    if _results_hook is not None:
        _results_hook(br)
    outs = [np.asarray(r["out"], np.float32) for r in br.results]
    bu = np.asarray(bu, np.float32)
    full = np.stack([outs[2 * b] + outs[2 * b + 1] for b in range(B)], axis=0)
    full += bu.reshape(1, 1, -1)
    return full.astype(np.float32)


# revision 40
# speedup vs baseline: 1.0120x; 1.0087x over previous
"""Bass/Tile kernel for nn_MCA (multi-head cross-attention), 8-core SPMD.

Sharding: batch B(4) x head-group(2) -> 8 cores. Core c handles batch
b = c//2 and heads [g*8, (g+1)*8) where g = c%2. Each core computes a
partial output (T, C) = y_g @ Wu[:, g-cols].T; host sums the two
head-group partials per batch and adds bu.

Single fused pipeline (all matmuls bf16 -> fp32 PSUM, uniform
(128,128) PE tile mode so the array never drains on a mode switch).
The exp stream on the Scalar engine is the bound (256 x [128,1024]
activations ~ 294us); every projection hides under it as paced PE
filler:

  prologue: only qT(qb0,m0) + kT(block0,m0).
  main loop over 256 groups: software-pipelined S(g+1) -> exp(g) ->
    V(g-1); filler thunks (remaining projections, prev q-block's output
    projection) drain between groups with a matmul budget.
  qb0's k-sweep is split into two half-sweeps over all head pairs
  (A: k-tiles 0-7, B: 8-15, partial o spilled to SBUF between) so the
  first sweep's new-k-tile consumption rate stays under the projection
  production rate and the exp stream never starves.
  S matmuls are zero-padded to K=128 (kTz: per-head slices, data on
  the head's partition half, zeros elsewhere).
  Rowsum via the ones-column trick (V stationary [128,65]); per head
  pair: batched DVE reciprocal, DMA hop to partition 0, gpsimd
  broadcast, DVE multiply into yT.
"""

import os
from contextlib import ExitStack

import numpy as np

_PROGRAM_CACHE = {}


def _imports():
    import concourse.bass as bass
    import concourse.tile as tile
    from concourse import bacc, mybir
    from concourse.bass_utils import run_bass_kernel_spmd

    return bass, tile, bacc, mybir, run_bass_kernel_spmd


def build_program(T=2048, C=1024, HLOC=8, n_cores=8):
    """Build + compile the per-core Tile program (SPMD; same for all cores)."""
    bass, tile, bacc, mybir, _ = _imports()
    BF16 = mybir.dt.bfloat16
    F32 = mybir.dt.float32
    AF = mybir.ActivationFunctionType
    ALU = mybir.AluOpType

    hd = 64
    DG = HLOC * hd            # head-group feature dim (512)
    P = 128
    KT = C // P               # contraction tiles for projections (8)
    MT = DG // P              # d-tiles (4)
    NBLK = 512                # t-block width for projections / q-blocks
    NB = T // NBLK            # 4
    KTT = T // P              # key tiles in attention (16)
    HP = MT                   # head pairs == d-tiles
    scale = 1.0 / np.sqrt(C)

    nc = bacc.Bacc("TRN2", target_bir_lowering=False, debug=False,
                   num_devices=n_cores)

    # host supplies pre-transposed layouts so every DMA is contiguous
    # per partition
    xqT = nc.dram_tensor("xqT", [C, T], BF16, kind="ExternalInput").ap()
    xkT = nc.dram_tensor("xkT", [C, T], BF16, kind="ExternalInput").ap()
    wqT = nc.dram_tensor("wqT", [C, DG], BF16, kind="ExternalInput").ap()
    wkT = nc.dram_tensor("wkT", [C, DG], BF16, kind="ExternalInput").ap()
    wvT = nc.dram_tensor("wvT", [C, DG], BF16, kind="ExternalInput").ap()
    wuT = nc.dram_tensor("wuT", [DG, C], BF16, kind="ExternalInput").ap()
    bq = nc.dram_tensor("bq", [P, MT], F32, kind="ExternalInput").ap()
    bk = nc.dram_tensor("bk", [P, MT], F32, kind="ExternalInput").ap()
    bv = nc.dram_tensor("bv", [1, DG], F32, kind="ExternalInput").ap()
    out = nc.dram_tensor("out", [T, C], F32, kind="ExternalOutput").ap()

    with tile.TileContext(nc) as tc, ExitStack() as ctx:
        const = ctx.enter_context(tc.tile_pool(name="const", bufs=1))
        persist = ctx.enter_context(tc.tile_pool(name="persist", bufs=1))
        xqpool = ctx.enter_context(tc.tile_pool(name="xqpool", bufs=2))
        xkpool = ctx.enter_context(tc.tile_pool(name="xkpool", bufs=3))
        # PSUM budget (8 banks): S 2x[128,2,512]=4, o 2x[65,512]=2,
        # proj 2x[128,512]=2
        ps_s = ctx.enter_context(tc.tile_pool(name="ps_s", bufs=2,
                                              space="PSUM"))
        ps_o = ctx.enter_context(tc.tile_pool(name="ps_o", bufs=2,
                                              space="PSUM"))
        ps_p = ctx.enter_context(tc.tile_pool(name="ps_p", bufs=2,
                                              space="PSUM"))
        ppool = ctx.enter_context(tc.tile_pool(name="ppool", bufs=3))
        epil = ctx.enter_context(tc.tile_pool(name="epil", bufs=4))
        outp = ctx.enter_context(tc.tile_pool(name="outp", bufs=2))

        # ---- constants / weights resident in SBUF ----
        wq_sb = persist.tile([P, KT, DG], BF16)
        wk_sb = persist.tile([P, KT, DG], BF16)
        wv_sb = persist.tile([P, KT, DG], BF16)
        wu_sb = persist.tile([P, MT, C], BF16)
        bq_sb = const.tile([P, MT], F32)
        bk_sb = const.tile([P, MT], F32)
        bv_bc = const.tile([P, DG], F32)
        # DMA in need-order: Q-projection inputs first so the PE starts
        # within a few us, output-projection weights last
        nc.sync.dma_start(out=wq_sb[:], in_=wqT.rearrange("(k p) d -> p k d", p=P))
        nc.sync.dma_start(out=bq_sb[:], in_=bq)

        # persistent activations
        qT_sb = persist.tile([P, MT, T], BF16)
        # kTz: one [128, T] slice per head, the head's 64 k-dims on its
        # partition half ((h%2)*64) and zeros on the other half, so S
        # matmuls run with K=128 (uniform tile mode, no PE drains).
        kTz_sb = persist.tile([P, HLOC, T], BF16)
        v_aug = persist.tile([P, KTT, HLOC * (hd + 1)], BF16)
        yT_sb = persist.tile([P, MT, T], BF16)

        xq_tiles = {}
        xk_tiles = {}
        proj_ps = {}

        def load_xq(nt):
            if nt not in xq_tiles:
                t = xqpool.tile([P, KT, NBLK], BF16, tag="xq", name="xq_t")
                nc.sync.dma_start(
                    out=t[:],
                    in_=xqT.rearrange("(k p) t -> p k t", p=P)[
                        :, :, nt * NBLK:(nt + 1) * NBLK])
                xq_tiles[nt] = t
            return xq_tiles[nt]

        def load_xk(nt):
            if nt not in xk_tiles:
                t = xkpool.tile([P, KT, NBLK], BF16, tag="xk", name="xk_t")
                nc.sync.dma_start(
                    out=t[:],
                    in_=xkT.rearrange("(k p) t -> p k t", p=P)[
                        :, :, nt * NBLK:(nt + 1) * NBLK])
                xk_tiles[nt] = t
            return xk_tiles[nt]

        # first x block + K-side weights next in the DMA queue
        load_xq(0)
        nc.sync.dma_start(out=wk_sb[:], in_=wkT.rearrange("(k p) d -> p k d", p=P))
        nc.sync.dma_start(out=bk_sb[:], in_=bk)
        load_xk(0)
        nc.sync.dma_start(out=wv_sb[:], in_=wvT.rearrange("(k p) d -> p k d", p=P))
        nc.sync.dma_start(out=bv_bc[:], in_=bv.partition_broadcast(P))
        nc.sync.dma_start(out=wu_sb[:], in_=wuT.rearrange("(k p) d -> p k d", p=P))

        # per-partition masks for the zero-padded kTz evictions: head 2m
        # keeps partitions 0:64, head 2m+1 keeps 64:128; the eviction
        # writes the full 128 partitions (data*mask + masked bias) so no
        # whole-tensor memset is needed (a long-running gpsimd memset
        # racing the evictions corrupts kTz).
        mask_lo = const.tile([P, 1], F32)
        mask_hi = const.tile([P, 1], F32)
        nc.vector.memset(mask_lo[0:hd, :], 1.0)
        nc.vector.memset(mask_lo[hd:P, :], 0.0)
        nc.vector.memset(mask_hi[0:hd, :], 0.0)
        nc.vector.memset(mask_hi[hd:P, :], 1.0)
        bk_lo = const.tile([P, MT], F32)
        bk_hi = const.tile([P, MT], F32)
        nc.vector.tensor_scalar(out=bk_lo[:], in0=bk_sb[:],
                                scalar1=mask_lo[:], scalar2=None,
                                op0=ALU.mult)
        nc.vector.tensor_scalar(out=bk_hi[:], in0=bk_sb[:],
                                scalar1=mask_hi[:], scalar2=None,
                                op0=ALU.mult)

        def emit_qT(nt, m, half):
            """Half of one Q-projection output tile (4 matmuls); the
            second half evicts with bias + 1/sqrt(C) scale."""
            tsl = slice(nt * NBLK, (nt + 1) * NBLK)
            xq_t = load_xq(nt)
            msl = slice(m * P, (m + 1) * P)
            if half == 0:
                proj_ps[("q", nt, m)] = ps_p.tile([P, NBLK], F32, tag="p",
                                                  name="psq")
            ps = proj_ps[("q", nt, m)]
            for k in range(4 * half, 4 * half + 4):
                nc.tensor.matmul(ps[:], wq_sb[:, k, msl], xq_t[:, k, :],
                                 start=(k == 0), stop=(k == KT - 1))
            if half == 1:
                del proj_ps[("q", nt, m)]
                nc.vector.tensor_scalar(
                    out=qT_sb[:, m, tsl], in0=ps[:],
                    scalar1=bq_sb[:, m:m + 1], scalar2=scale,
                    op0=ALU.add, op1=ALU.mult)

        def emit_kT(nt, m, half):
            """Half of one K-projection d-pair tile; the second half
            evicts into the two per-head zero-padded kTz slices."""
            tsl = slice(nt * NBLK, (nt + 1) * NBLK)
            xk_t = load_xk(nt)
            msl = slice(m * P, (m + 1) * P)
            if half == 0:
                proj_ps[("k", nt, m)] = ps_p.tile([P, NBLK], F32, tag="p",
                                                  name="psk")
            ps = proj_ps[("k", nt, m)]
            for k in range(4 * half, 4 * half + 4):
                nc.tensor.matmul(ps[:], wk_sb[:, k, msl], xk_t[:, k, :],
                                 start=(k == 0), stop=(k == KT - 1))
            if half == 1:
                del proj_ps[("k", nt, m)]
                nc.vector.tensor_scalar(
                    out=kTz_sb[:, 2 * m, tsl], in0=ps[:],
                    scalar1=mask_lo[:], scalar2=bk_lo[:, m:m + 1],
                    op0=ALU.mult, op1=ALU.add)
                nc.vector.tensor_scalar(
                    out=kTz_sb[:, 2 * m + 1, tsl], in0=ps[:],
                    scalar1=mask_hi[:], scalar2=bk_hi[:, m:m + 1],
                    op0=ALU.mult, op1=ALU.add)

        def emit_v(nt, m, half):
            """Half of one V-projection t-subtile (all DG cols)."""
            tidx = nt * (NBLK // P) + m
            msl = slice(m * P, (m + 1) * P)
            xk_t = load_xk(nt)
            if half == 0:
                proj_ps[("v", nt, m)] = ps_p.tile([P, DG], F32, tag="p",
                                                  name="psv")
            ps = proj_ps[("v", nt, m)]
            for k in range(4 * half, 4 * half + 4):
                nc.tensor.matmul(ps[:], xk_t[:, k, msl], wv_sb[:, k, :],
                                 start=(k == 0), stop=(k == KT - 1))
            if half == 1:
                del proj_ps[("v", nt, m)]
                v_row = v_aug[:, tidx]
                for h in range(HLOC):
                    nc.vector.tensor_add(
                        v_row[:, h * (hd + 1):h * (hd + 1) + hd],
                        ps[:, h * hd:(h + 1) * hd],
                        bv_bc[:, h * hd:(h + 1) * hd])
                ones_view = v_row.rearrange("p (h e) -> p h e",
                                            e=hd + 1)[:, :, hd:hd + 1]
                nc.vector.memset(ones_view, 1.0)

        def emit_outproj(qt, jt):
            """One [128 t, 512 c] tile of the output projection."""
            qsl = slice(qt * P, (qt + 1) * P)
            jsl = slice(jt * NBLK, (jt + 1) * NBLK)
            ps = ps_p.tile([P, NBLK], F32, tag="p", name="pso")
            for dt in range(MT):
                nc.tensor.matmul(ps[:], yT_sb[:, dt, qsl],
                                 wu_sb[:, dt, jsl],
                                 start=(dt == 0), stop=(dt == MT - 1))
            o_sb = outp.tile([P, NBLK], F32, tag="osb", name="osb")
            nc.vector.tensor_copy(o_sb[:], ps[:])
            nc.sync.dma_start(out=out[qsl, jsl], in_=o_sb[:])

        # ---- filler stream: (n_matmuls, thunk) pairs, paced by budget ----
        filler = []

        def drain_filler(mm_budget):
            while mm_budget > 0 and filler:
                n_mm, fn = filler.pop(0)
                fn()
                mm_budget -= n_mm

        def q_thunk(fn, *args):
            filler.append((4, lambda a=args: fn(*a)))

        def queue_kT(nt, m):
            for half in range(2):
                q_thunk(emit_kT, nt, m, half)

        def queue_qT_m(nt, m):
            for half in range(2):
                q_thunk(emit_qT, nt, m, half)

        def queue_v_block(nt):
            for m in range(NBLK // P):
                for half in range(2):
                    q_thunk(emit_v, nt, m, half)

        def queue_outproj(qb):
            for qt in range(qb * NBLK // P, (qb + 1) * NBLK // P):
                for jt in range(C // NBLK):
                    filler.append(
                        (4, lambda qt=qt, jt=jt: emit_outproj(qt, jt)))

        # ---- prologue ----
        # The PE pulls LDWEIGHTS up to ~64 instructions ahead of in-flight
        # matmuls, so stationary operands (kTz, v_aug) must be WRITTEN with
        # that much emission distance before their first consuming matmul.
        # Prologue covers the first half-sweep's k-tiles generously.
        for half in range(2):
            emit_qT(0, 0, half)
        for nt in (0, 1):
            for half in range(2):
                emit_kT(nt, 0, half)
        for nt in (0, 1):
            for m in range(NBLK // P):
                for half in range(2):
                    emit_v(nt, m, half)

        # deadline-ordered filler for the A/B q-block-0 schedule: m-sets
        # for blocks 0-1 + qT0 lead hp1/2/3 of the A half-sweep (g=8/16/
        # 24); blocks 2-3 m0 + v2/v3 lead the B half-sweep (g>=32); late
        # m-slices of blocks 2-3 lead their B head pairs (g=40/48/56);
        # qT(qb1) by g=63. The budget paces these so the exp stream is fed
        # while the surplus spills into qb1-3 where the PE has spare time.
        for m in range(1, MT):
            queue_kT(0, m)
            queue_kT(1, m)
            queue_qT_m(0, m)
        queue_kT(2, 0)
        queue_kT(3, 0)
        queue_v_block(2)
        queue_v_block(3)
        queue_kT(2, 1)
        queue_kT(3, 1)
        for m in range(MT):
            queue_qT_m(1, m)
        for m in range(2, MT):
            queue_kT(2, m)
            queue_kT(3, m)


        # ---- group list ----
        # entry: (qb, hp, h2, ktp, lo, hi, final)
        groups = []
        qb_first_group = set()
        for qb in range(NB):
            qb_first_group.add(len(groups))
            phases = [(0, 4, False), (4, 8, True)] if qb == 0 else [(0, 8, True)]
            for lo, hi, final in phases:
                for hp in range(HP):
                    for ktp in range(lo, hi):
                        for h2 in range(2):
                            groups.append((qb, hp, h2, ktp, lo, hi, final))
        NG = len(groups)

        def emit_S(g):
            qb, hp, h2, ktp, lo, hi, final = groups[g]
            h = 2 * hp + h2
            qsl = slice(qb * NBLK, (qb + 1) * NBLK)
            s = ps_s.tile([P, 2, NBLK], F32, tag="s", name="s_ps")
            for j in range(2):
                kt = 2 * ktp + j
                ksl = slice(kt * P, (kt + 1) * P)
                nc.tensor.matmul(s[:, j, :], kTz_sb[:, h, ksl],
                                 qT_sb[:, hp, qsl], start=True, stop=True)
            return s

        def emit_exp(g, s):
            p = ppool.tile([P, 2, NBLK], BF16, tag="p", name="p_sb")
            nc.scalar.activation(p[:], s[:], AF.Exp)
            return p

        o_tiles = {}    # (qb, h) -> psum tile accumulating [65, NBLK]
        acc_tiles = {}  # (qb, h) -> SBUF partial o from the A half-sweep
        norm_sbs = {}   # (qb, h) -> o_sb copy awaiting the pair recip
        rs_tiles = {}   # (qb, hp) -> [2, NBLK] gathered rowsums

        def emit_norm_copy(qb, h, o_t):
            """Evict o psum (adding the A-phase partial if any); gather the
            rowsum row into the per-pair batch via DMA."""
            hp = h // 2
            if (qb, hp) not in rs_tiles:
                rs_tiles[(qb, hp)] = epil.tile([2, NBLK], F32, tag="rs",
                                               bufs=2, name="rs_all")
            o_sb = epil.tile([hd + 1, NBLK], F32, tag="o_sb", bufs=3,
                             name="o_sb")
            if (qb, h) in acc_tiles:
                nc.vector.tensor_add(o_sb[:], o_t[:],
                                     acc_tiles.pop((qb, h))[:])
            else:
                nc.vector.tensor_copy(o_sb[:], o_t[:])
            nc.sync.dma_start(out=rs_tiles[(qb, hp)][h % 2:h % 2 + 1, :],
                              in_=o_sb[hd:hd + 1, :])
            norm_sbs[(qb, h)] = o_sb

        def emit_norm_finish(qb, hp):
            """Pair reciprocal, then per-head broadcast + multiply into
            yT."""
            qsl = slice(qb * NBLK, (qb + 1) * NBLK)
            recip_t = epil.tile([2, NBLK], F32, tag="recip", bufs=2,
                                name="recip_t")
            nc.vector.reciprocal(recip_t[:], rs_tiles.pop((qb, hp))[:])
            for h2 in range(2):
                h = 2 * hp + h2
                hrow = slice(0, hd) if h2 == 0 else slice(hd, P)
                rtmp = epil.tile([1, NBLK], F32, tag="rtmp", bufs=2,
                                 name="rtmp")
                nc.sync.dma_start(out=rtmp[:], in_=recip_t[h2:h2 + 1, :])
                bcast = epil.tile([hd, NBLK], F32, tag="bcast", bufs=2,
                                  name="bcast")
                nc.gpsimd.partition_broadcast(bcast[:], rtmp[:])
                nc.vector.tensor_mul(yT_sb[hrow, hp, qsl],
                                     norm_sbs.pop((qb, h))[0:hd, :],
                                     bcast[:])

        def emit_V(g, p):
            qb, hp, h2, ktp, lo, hi, final = groups[g]
            h = 2 * hp + h2
            key = (qb, h)
            if ktp == lo:
                o_tiles[key] = ps_o.tile([hd + 1, NBLK], F32, tag="o",
                                         name="o_ps")
            o_t = o_tiles[key]
            for j in range(2):
                kt = 2 * ktp + j
                nc.tensor.matmul(
                    o_t[:], v_aug[:, kt, h * (hd + 1):(h + 1) * (hd + 1)],
                    p[:, j, :], start=(ktp == lo and j == 0),
                    stop=(ktp == hi - 1 and j == 1))
            if ktp == hi - 1:
                o_t = o_tiles.pop(key)
                if not final:
                    acc = epil.tile([hd + 1, NBLK], F32, tag="oacc",
                                    bufs=8, name="oacc")
                    nc.vector.tensor_copy(acc[:], o_t[:])
                    acc_tiles[key] = acc
                else:
                    emit_norm_copy(qb, h, o_t)
                    if h2 == 1:
                        emit_norm_finish(qb, hp)

        # software pipeline: S(g+1) ahead of exp(g) ahead of V(g-1)
        s_tiles = {0: emit_S(0)}
        p_tiles = {}
        for g in range(NG):
            qb, hp, h2, ktp, lo, hi, final = groups[g]
            if g in qb_first_group:
                if 1 <= qb + 1 < NB and qb >= 1:
                    for m in range(MT):
                        queue_qT_m(qb + 1, m)
                if qb >= 1:
                    queue_outproj(qb - 1)
            if g + 1 < NG:
                s_tiles[g + 1] = emit_S(g + 1)
            p_tiles[g] = emit_exp(g, s_tiles.pop(g))
            if g - 1 >= 0:
                emit_V(g - 1, p_tiles.pop(g - 1))
            # deadline-paced filler drain: just enough to meet projection
            # deadlines early, then the uniform leftover rate (~2mm/group)
            if g < 8:
                b = 4
            elif g < 32:
                b = 5
            else:
                b = 2
            drain_filler(b)
        emit_V(NG - 1, p_tiles.pop(NG - 1))

        # ---- epilogue: last q-block's output projection + leftovers ----
        queue_outproj(NB - 1)
        drain_filler(10 ** 9)

    nc.compile()
    return nc


def _get_program():
    key = "main"
    if key not in _PROGRAM_CACHE:
        _PROGRAM_CACHE[key] = build_program()
    return _PROGRAM_CACHE[key]


def make_in_maps(x1, x2, Wq, bq, Wk, bk, Wv, bv, Wu, bu, n_cores=8):
    import ml_dtypes
    bf16 = ml_dtypes.bfloat16
    T, B, C = x1.shape
    DG = C // 2  # head-group feature dim (8 heads x 64)
    P, KT, NBLK, NB = 128, C // 128, 512, T // 512
    x1 = np.asarray(x1, np.float32)
    x2 = np.asarray(x2, np.float32)

    def prep_w(w):  # [C, M] -> [128, KT, M]: partition p <- row k*128+p
        return np.ascontiguousarray(
            w.reshape(KT, P, -1).transpose(1, 0, 2)).astype(bf16)

    def prep_x(xT):  # [C, T] -> [NB, 128, KT, NBLK]
        return np.ascontiguousarray(
            xT.reshape(KT, P, NB, NBLK).transpose(2, 1, 0, 3)).astype(bf16)

    in_maps = []
    for core in range(n_cores):
        b, g = core // 2, core % 2
        gs = slice(g * DG, (g + 1) * DG)
        in_maps.append({
            "xqT": np.ascontiguousarray(x1[:, b, :].T).astype(bf16),
            "xkT": np.ascontiguousarray(x2[:, b, :].T).astype(bf16),
            "wqT": np.ascontiguousarray(np.asarray(Wq, np.float32)[gs, :].T).astype(bf16),
            "wkT": np.ascontiguousarray(np.asarray(Wk, np.float32)[gs, :].T).astype(bf16),
            "wvT": np.ascontiguousarray(np.asarray(Wv, np.float32)[gs, :].T).astype(bf16),
            "wuT": np.ascontiguousarray(np.asarray(Wu, np.float32)[:, gs].T).astype(bf16),
            "bq": np.ascontiguousarray(
                np.asarray(bq, np.float32)[gs].reshape(-1, P).T),
            "bk": np.ascontiguousarray(
                np.asarray(bk, np.float32)[gs].reshape(-1, P).T),
            "bv": np.asarray(bv, np.float32)[gs].reshape(1, DG),
        })
    return in_maps


def kernel(x1, x2, Wq, bq, Wk, bk, Wv, bv, Wu, bu, _results_hook=None):
    _, _, _, _, run_bass_kernel_spmd = _imports()
    T, B, C = x1.shape
    nc = _get_program()
    in_maps = make_in_maps(x1, x2, Wq, bq, Wk, bk, Wv, bv, Wu, bu)
    br = run_bass_kernel_spmd(nc, in_maps, list(range(8)))
    if _results_hook is not None:
        _results_hook(br)
    outs = [np.asarray(r["out"], np.float32) for r in br.results]
    bu = np.asarray(bu, np.float32)
    full = np.stack([outs[2 * b] + outs[2 * b + 1] for b in range(B)], axis=0)
    full += bu.reshape(1, 1, -1)
    return full.astype(np.float32)
